# revision 16
# baseline (speedup 1.0000x reference)
"""MoE MLP (9 experts, top-2 routing) on 8 TRN2 NeuronCores.

Strategy: expert-parallel. The router (tiny) runs on host CPU with the exact
jax ops of the reference so top-2 selection matches bitwise. Tokens are
gathered per expert on host; the largest expert is split across all 8 cores
(slot B), each core additionally owns one of the remaining 8 experts
(slot A). Every core runs the same SPMD Bass program (shapes baked from the
actual routing at call time): gate/up matmuls (bf16, fp32 PSUM), silu*up,
down matmul, all with features on partitions and tokens on the free dim so
no transposes are needed. Host applies combine weights and scatter-adds.

Two schedule details: ~16 dummy warm-up matmuls on a zeroed tile keep the
PE busy from program start so the HAM clock gate (K=4/8 cold -> 8/8 warm,
~3.4us busy window) flips during the initial DMA fill instead of ~13us
into real work; and the output is stored as bf16, halving the tail DMA.
"""

import os

# The tunneled NeuronCores can be left wedged (NRT_EXEC_UNIT_UNRECOVERABLE)
# by a previous process; resetting cores at NRT init makes runs reliable.
os.environ.setdefault("NEURON_RT_RESET_CORES", "1")

import numpy as np
import ml_dtypes

import jax
import jax.numpy as jnp

import concourse.bass as bass
import concourse.mybir as mybir
import concourse.tile as tile
from concourse import bacc
from concourse.bass_utils import run_bass_kernel_spmd
from concourse.tile_rust import add_dep_helper

BF16 = ml_dtypes.bfloat16
H = 1024
I = 2816
E = 9
TOPK = 2
NCORES = 8
P = 128
HK = H // P       # 8 partition-tiles over H
IK = I // P       # 22 partition-tiles over I
NT = 512          # token tile (PSUM bank = 512 fp32)
NWARM = 9         # dummy matmuls that trip the HAM clock gate early
WARMFD = 384

LAST_EXEC_NS = None          # set when BASS_TRACE=1 (read by test harness)
_PROGRAM_CACHE = {}


def _route(x, Wr):
    """Router on jax-CPU, eager, with the reference's exact op sequence."""
    cpu = jax.devices("cpu")[0]
    with jax.default_device(cpu):
        xj = jnp.asarray(np.asarray(x))
        wj = jnp.asarray(np.asarray(Wr))
        logits = jnp.einsum("bsh,he->bse", xj, wj)
        probs = jax.nn.softmax(logits, axis=-1)
        topk_w, topk_idx = jax.lax.top_k(probs, TOPK)
        topk_w = topk_w / jnp.sum(topk_w, axis=-1, keepdims=True)
    T = x.shape[0] * x.shape[1]
    return (np.asarray(topk_idx).reshape(T, TOPK),
            np.asarray(topk_w).astype(np.float32).reshape(T, TOPK))


def _token_units(CA, CB):
    """(slot, col0, ncols, localcol0) units covering [0, CA+CB)."""
    units = []
    for c0 in range(0, CA, NT):
        units.append((0, c0, min(NT, CA - c0), c0))
    for c0 in range(0, CB, NT):
        units.append((1, CA + c0, min(NT, CB - c0), c0))
    return units


def _build_program(CA, CB):
    C = CA + CB
    nc = bacc.Bacc("TRN2", target_bir_lowering=False, debug=False,
                   num_devices=NCORES)
    bf = mybir.dt.bfloat16
    f32 = mybir.dt.float32
    xt_d = nc.dram_tensor("xt", [HK, P, C], bf, kind="ExternalInput")
    wg_d = nc.dram_tensor("wg", [2, IK, P, HK, P], bf, kind="ExternalInput")
    wu_d = nc.dram_tensor("wu", [2, IK, P, HK, P], bf, kind="ExternalInput")
    wd_d = nc.dram_tensor("wd", [2, HK, P, IK, P], bf, kind="ExternalInput")
    y_d = nc.dram_tensor("y", [HK, P, C], bf, kind="ExternalOutput")

    units = _token_units(CA, CB)

    with tile.TileContext(nc) as tc:
        with (
            tc.tile_pool(name="warm", bufs=1) as warm,
            tc.tile_pool(name="xpool", bufs=1) as xpool,
            tc.tile_pool(name="hpool", bufs=1) as hpool,
            tc.tile_pool(name="wpool", bufs=2) as wpool,
            tc.tile_pool(name="wdpool", bufs=2) as wdpool,
            tc.tile_pool(name="gpool", bufs=3) as gpool,
            tc.tile_pool(name="ypool", bufs=3) as ypool,
            tc.tile_pool(name="ps1", bufs=3, space="PSUM") as ps1,
            tc.tile_pool(name="ps2", bufs=2, space="PSUM") as ps2,
        ):
            # HAM warm-up: PE busy from t~0 while DMAs fill SBUF, so the
            # clock gate is at K=8/8 before the first real matmul. The
            # dummy PSUM tile borrows the phase-2 pool (WAW only; phase 2
            # starts long after these retire).
            wz = warm.tile([P, WARMFD], bf, tag="wz", name="wz")
            nc.vector.memset(wz[:], 0)
            pw = ps2.tile([P, NT], f32, tag="pd", name="pw")[:, :WARMFD]
            for _ in range(NWARM):
                nc.tensor.matmul(pw, wz[:, :P], wz, start=True, stop=True)

            # resident tokens: one tile per H k-tile so the k-th matmul of
            # the first accumulation group only waits on its own DMA
            xts = []
            with tc.high_priority():
                for k in range(HK):
                    xk = xpool.tile([P, C], bf, tag=f"xt{k}", name=f"xt{k}")
                    nc.sync.dma_start(xk[:], xt_d[k])
                    xts.append(xk)
            hid = [hpool.tile([P, IK, CA], bf, tag="hidA", name="hidA"),
                   hpool.tile([P, IK, CB], bf, tag="hidB", name="hidB")]

            # phase 1: gate/up + silu*up, streaming Wg/Wu by I-tile
            p1_marker = None
            for i in range(IK):
                wgt, wut = [], []
                for s in (0, 1):
                    g = wpool.tile([P, HK, P], bf, tag=f"wg{s}", name=f"wg{s}")
                    dg = nc.sync.dma_start(g[:], wg_d[s, i])
                    u = wpool.tile([P, HK, P], bf, tag=f"wu{s}", name=f"wu{s}")
                    du = nc.sync.dma_start(u[:], wu_d[s, i])
                    wgt.append(g)
                    wut.append(u)
                    if i == 0:
                        # keep all first-i-tile weight loads ahead of the
                        # prefetch stream (the k-outer i=0 loop needs every
                        # slot's weights almost immediately)
                        dg.ins.bass_priority = 0
                        du.ins.bass_priority = 0
                if i == 0:
                    # k-outer for the first i-tile: all unit accumulators
                    # live (6 PSUM banks), 6 matmuls per arriving xt k-tile,
                    # so compute tracks the DMA fill instead of stalling on
                    # each k in turn
                    pgs = [ps1.tile([P, NT], f32, tag="pg", name="pg")[:, :n]
                           for (s, c0, n, lc) in units]
                    pus = [ps1.tile([P, NT], f32, tag="pu", name="pu")[:, :n]
                           for (s, c0, n, lc) in units]
                    for k in range(HK):
                        for ui, (s, c0, n, lc) in enumerate(units):
                            nc.tensor.matmul(pgs[ui], wgt[s][:, k, :],
                                             xts[k][:, c0:c0 + n],
                                             start=(k == 0),
                                             stop=(k == HK - 1))
                            nc.tensor.matmul(pus[ui], wut[s][:, k, :],
                                             xts[k][:, c0:c0 + n],
                                             start=(k == 0),
                                             stop=(k == HK - 1))
                    for ui, (s, c0, n, lc) in enumerate(units):
                        gt = gpool.tile([P, NT], bf, tag="gt", name="gt")[:, :n]
                        nc.scalar.activation(gt, pgs[ui],
                                             mybir.ActivationFunctionType.Silu)
                        nc.vector.tensor_mul(hid[s][:, i, lc:lc + n],
                                             gt, pus[ui])
                    continue
                for (s, c0, n, lc) in units:
                    pg = ps1.tile([P, NT], f32, tag="pg", name="pg")[:, :n]
                    pu = ps1.tile([P, NT], f32, tag="pu", name="pu")[:, :n]
                    for k in range(HK):
                        mm = nc.tensor.matmul(pg, wgt[s][:, k, :],
                                              xts[k][:, c0:c0 + n],
                                              start=(k == 0), stop=(k == HK - 1))
                        if i == 2 and p1_marker is None:
                            p1_marker = mm
                    for k in range(HK):
                        nc.tensor.matmul(pu, wut[s][:, k, :],
                                         xts[k][:, c0:c0 + n],
                                         start=(k == 0), stop=(k == HK - 1))
                    gt = gpool.tile([P, NT], bf, tag="gt", name="gt")[:, :n]
                    nc.scalar.activation(gt, pg,
                                         mybir.ActivationFunctionType.Silu)
                    nc.vector.tensor_mul(hid[s][:, i, lc:lc + n], gt, pu)

            # phase 2: down proj, streaming Wd by H-tile
            for j in range(HK):
                wdt = []
                for s in (0, 1):
                    d = wdpool.tile([P, IK, P], bf, tag=f"wd{s}", name=f"wd{s}")
                    dd = nc.sync.dma_start(d[:], wd_d[s, j])
                    if j < 2 and p1_marker is not None:
                        # keep the big Wd prefetches out of the startup
                        # window where they compete with first-needed DMAs
                        add_dep_helper(p1_marker.ins, dd.ins, sync=False,
                                       reason="delay wd prefetch")
                    wdt.append(d)
                for (s, c0, n, lc) in units:
                    pd = ps2.tile([P, NT], f32, tag="pd", name="pd")[:, :n]
                    for i in range(IK):
                        nc.tensor.matmul(pd, wdt[s][:, i, :],
                                         hid[s][:, i, lc:lc + n],
                                         start=(i == 0), stop=(i == IK - 1))
                    yt = ypool.tile([P, NT], bf, tag="yt", name="yt")[:, :n]
                    nc.vector.tensor_copy(yt, pd)
                    nc.sync.dma_start(y_d[j, :, c0:c0 + n], yt)

    nc.compile()
    return nc


def _pack_gateup(w):        # [H, I] -> [IK, P(ki), HK, P(ii)] contiguous
    return np.ascontiguousarray(
        w.reshape(HK, P, IK, P).transpose(2, 1, 0, 3))


def _pack_down(w):          # [I, H] -> [HK, P(ii), IK, P(jj)] contiguous
    return np.ascontiguousarray(
        w.reshape(IK, P, HK, P).transpose(2, 1, 0, 3))


def kernel(x, Wr, Wg, Wu, Wd):
    global LAST_EXEC_NS
    x = np.asarray(x)
    B, S, _ = x.shape
    T = B * S
    xf = np.asarray(x, dtype=np.float32).reshape(T, H)

    idx, w = _route(x, Wr)

    # per-expert token lists and combine weights
    toks, cws = [], []
    for e in range(E):
        m = idx == e
        te = np.nonzero(m.any(axis=1))[0]
        toks.append(te)
        cws.append((w * m).sum(axis=1)[te].astype(np.float32))
    counts = np.array([len(t) for t in toks])

    # Scheme "o" (original): biggest expert split over all 8 B-bins, the
    # other 8 each own an A-bin. Scheme "n": biggest expert over 7 B-bins,
    # 2nd-biggest gets core 7's A-bin plus its B-bin for the overflow --
    # C drops by ~9 tokens for these counts. Pick whichever packs tighter.
    order = [int(e) for e in np.argsort(-counts)]
    big1, big2, rest = order[0], order[1], order[2:]
    CA_n = max(2, int(counts[rest].max()))
    CB_n = max(2, int(max(-(-int(counts[big1]) // (NCORES - 1)),
                          int(counts[big2]) - CA_n)))
    CA_o = max(2, int(max(counts[e] for e in order[1:])))
    CB_o = max(2, int(-(-int(counts[big1]) // NCORES)))

    def chunk(e, lo, hi):
        return (e, toks[e][lo:hi], cws[e][lo:hi])

    if CA_n + CB_n < CA_o + CB_o:
        CA, CB = CA_n, CB_n
        slotA = [chunk(rest[c], 0, CA) for c in range(NCORES - 1)]
        slotA.append(chunk(big2, 0, CA))
        slotB = [chunk(big1, c * CB, (c + 1) * CB)
                 for c in range(NCORES - 1)]
        slotB.append(chunk(big2, CA, CA + CB))
    else:
        CA, CB = CA_o, CB_o
        slotA = [chunk(order[1 + c], 0, CA) for c in range(NCORES)]
        slotB = [chunk(big1, c * CB, (c + 1) * CB) for c in range(NCORES)]
    C = CA + CB

    key = (CA, CB)
    if key not in _PROGRAM_CACHE:
        _PROGRAM_CACHE[key] = _build_program(CA, CB)
    nc = _PROGRAM_CACHE[key]

    packs = {}

    def packs_of(e):
        if e not in packs:
            packs[e] = (_pack_gateup(np.asarray(Wg[e], dtype=BF16)),
                        _pack_gateup(np.asarray(Wu[e], dtype=BF16)),
                        _pack_down(np.asarray(Wd[e], dtype=BF16)))
        return packs[e]

    in_maps = []
    for c in range(NCORES):
        (ea, ta, _), (eb, tb_c, _) = slotA[c], slotB[c]
        pa, pb = packs_of(ea), packs_of(eb)
        xt = np.zeros((H, C), dtype=BF16)
        if len(ta):
            xt[:, :len(ta)] = xf[ta].T
        if len(tb_c):
            xt[:, CA:CA + len(tb_c)] = xf[tb_c].T
        in_maps.append({
            "xt": np.ascontiguousarray(xt.reshape(HK, P, C)),
            "wg": np.stack([pa[0], pb[0]]),
            "wu": np.stack([pa[1], pb[1]]),
            "wd": np.stack([pa[2], pb[2]]),
        })

    res = run_bass_kernel_spmd(nc, in_maps, core_ids=list(range(NCORES)))
    LAST_EXEC_NS = res.exec_time_ns

    out = np.zeros((T, H), dtype=np.float32)
    for c in range(NCORES):
        y = np.asarray(res.results[c]["y"], dtype=np.float32)
        y = y.reshape(H, C).T                 # [C, H]
        (_, ta, wa), (_, tb_c, wb) = slotA[c], slotB[c]
        if len(ta):
            out[ta] += y[:len(ta)] * wa[:, None]
        if len(tb_c):
            out[tb_c] += y[CA:CA + len(tb_c)] * wb[:, None]

    return out.reshape(B, S, H)


# revision 17
# speedup vs baseline: 1.0098x; 1.0098x over previous
"""MoE MLP (9 experts, top-2 routing) on 8 TRN2 NeuronCores.

Strategy: expert-parallel. The router (tiny) runs on host CPU with the exact
jax ops of the reference so top-2 selection matches bitwise. Tokens are
gathered per expert on host; the largest expert is split across all 8 cores
(slot B), each core additionally owns one of the remaining 8 experts
(slot A). Every core runs the same SPMD Bass program (shapes baked from the
actual routing at call time): gate/up matmuls (bf16, fp32 PSUM), silu*up,
down matmul, all with features on partitions and tokens on the free dim so
no transposes are needed. Host applies combine weights and scatter-adds.

Two schedule details: ~16 dummy warm-up matmuls on a zeroed tile keep the
PE busy from program start so the HAM clock gate (K=4/8 cold -> 8/8 warm,
~3.4us busy window) flips during the initial DMA fill instead of ~13us
into real work; and the output is stored as bf16, halving the tail DMA.
"""

import os

# The tunneled NeuronCores can be left wedged (NRT_EXEC_UNIT_UNRECOVERABLE)
# by a previous process; resetting cores at NRT init makes runs reliable.
os.environ.setdefault("NEURON_RT_RESET_CORES", "1")

import numpy as np
import ml_dtypes

import jax
import jax.numpy as jnp

import concourse.bass as bass
import concourse.mybir as mybir
import concourse.tile as tile
from concourse import bacc
from concourse.bass_utils import run_bass_kernel_spmd
from concourse.tile_rust import add_dep_helper

BF16 = ml_dtypes.bfloat16
H = 1024
I = 2816
E = 9
TOPK = 2
NCORES = 8
P = 128
HK = H // P       # 8 partition-tiles over H
IK = I // P       # 22 partition-tiles over I
NT = 512          # token tile (PSUM bank = 512 fp32)
NWARM = 9         # dummy matmuls that trip the HAM clock gate early
WARMFD = 384

LAST_EXEC_NS = None          # set when BASS_TRACE=1 (read by test harness)
_PROGRAM_CACHE = {}


def _route(x, Wr):
    """Router on jax-CPU, eager, with the reference's exact op sequence."""
    cpu = jax.devices("cpu")[0]
    with jax.default_device(cpu):
        xj = jnp.asarray(np.asarray(x))
        wj = jnp.asarray(np.asarray(Wr))
        logits = jnp.einsum("bsh,he->bse", xj, wj)
        probs = jax.nn.softmax(logits, axis=-1)
        topk_w, topk_idx = jax.lax.top_k(probs, TOPK)
        topk_w = topk_w / jnp.sum(topk_w, axis=-1, keepdims=True)
    T = x.shape[0] * x.shape[1]
    return (np.asarray(topk_idx).reshape(T, TOPK),
            np.asarray(topk_w).astype(np.float32).reshape(T, TOPK))


def _token_units(CA, CB):
    """(slot, col0, ncols, localcol0) units covering [0, CA+CB)."""
    units = []
    for c0 in range(0, CA, NT):
        units.append((0, c0, min(NT, CA - c0), c0))
    for c0 in range(0, CB, NT):
        units.append((1, CA + c0, min(NT, CB - c0), c0))
    return units


def _build_program(CA, CB):
    C = CA + CB
    nc = bacc.Bacc("TRN2", target_bir_lowering=False, debug=False,
                   num_devices=NCORES)
    bf = mybir.dt.bfloat16
    f32 = mybir.dt.float32
    xt_d = nc.dram_tensor("xt", [HK, P, C], bf, kind="ExternalInput")
    wg_d = nc.dram_tensor("wg", [2, IK, P, HK, P], bf, kind="ExternalInput")
    wu_d = nc.dram_tensor("wu", [2, IK, P, HK, P], bf, kind="ExternalInput")
    wd_d = nc.dram_tensor("wd", [2, HK, P, IK, P], bf, kind="ExternalInput")
    y_d = nc.dram_tensor("y", [HK, P, C], bf, kind="ExternalOutput")

    units = _token_units(CA, CB)

    with tile.TileContext(nc) as tc:
        with (
            tc.tile_pool(name="warm", bufs=1) as warm,
            tc.tile_pool(name="xpool", bufs=1) as xpool,
            tc.tile_pool(name="hpool", bufs=1) as hpool,
            tc.tile_pool(name="wpool", bufs=2) as wpool,
            tc.tile_pool(name="wdpool", bufs=2) as wdpool,
            tc.tile_pool(name="gpool", bufs=3) as gpool,
            tc.tile_pool(name="ypool", bufs=3) as ypool,
            tc.tile_pool(name="ps1", bufs=3, space="PSUM") as ps1,
            tc.tile_pool(name="ps2", bufs=2, space="PSUM") as ps2,
        ):
            # HAM warm-up: PE busy from t~0 while DMAs fill SBUF, so the
            # clock gate is at K=8/8 before the first real matmul. The
            # dummy PSUM tile borrows the phase-2 pool (WAW only; phase 2
            # starts long after these retire).
            wz = warm.tile([P, WARMFD], bf, tag="wz", name="wz")
            nc.vector.memset(wz[:], 0)
            pw = ps2.tile([P, NT], f32, tag="pd", name="pw")[:, :WARMFD]
            for _ in range(NWARM):
                nc.tensor.matmul(pw, wz[:, :P], wz, start=True, stop=True)

            # resident tokens: one tile per H k-tile so the k-th matmul of
            # the first accumulation group only waits on its own DMA
            xts = []
            with tc.high_priority():
                for k in range(HK):
                    xk = xpool.tile([P, C], bf, tag=f"xt{k}", name=f"xt{k}")
                    nc.sync.dma_start(xk[:], xt_d[k])
                    xts.append(xk)
            hid = [hpool.tile([P, IK, CA], bf, tag="hidA", name="hidA"),
                   hpool.tile([P, IK, CB], bf, tag="hidB", name="hidB")]

            # phase 1: gate/up + silu*up, streaming Wg/Wu by I-tile
            p1_marker = None
            for i in range(IK):
                wgt, wut = [], []
                for s in (0, 1):
                    g = wpool.tile([P, HK, P], bf, tag=f"wg{s}", name=f"wg{s}")
                    dg = nc.sync.dma_start(g[:], wg_d[s, i])
                    u = wpool.tile([P, HK, P], bf, tag=f"wu{s}", name=f"wu{s}")
                    nc.sync.dma_start(u[:], wu_d[s, i])
                    wgt.append(g)
                    wut.append(u)
                    if i == 0 and s == 0:
                        # keep the first-needed weight load ahead of prefetch
                        dg.ins.bass_priority = 0
                for (s, c0, n, lc) in units:
                    pg = ps1.tile([P, NT], f32, tag="pg", name="pg")[:, :n]
                    pu = ps1.tile([P, NT], f32, tag="pu", name="pu")[:, :n]
                    for k in range(HK):
                        mm = nc.tensor.matmul(pg, wgt[s][:, k, :],
                                              xts[k][:, c0:c0 + n],
                                              start=(k == 0), stop=(k == HK - 1))
                        if i == 2 and p1_marker is None:
                            p1_marker = mm
                    for k in range(HK):
                        nc.tensor.matmul(pu, wut[s][:, k, :],
                                         xts[k][:, c0:c0 + n],
                                         start=(k == 0), stop=(k == HK - 1))
                    gt = gpool.tile([P, NT], bf, tag="gt", name="gt")[:, :n]
                    nc.scalar.activation(gt, pg,
                                         mybir.ActivationFunctionType.Silu)
                    nc.vector.tensor_mul(hid[s][:, i, lc:lc + n], gt, pu)

            # phase 2: down proj, streaming Wd by H-tile
            for j in range(HK):
                wdt = []
                for s in (0, 1):
                    d = wdpool.tile([P, IK, P], bf, tag=f"wd{s}", name=f"wd{s}")
                    dd = nc.sync.dma_start(d[:], wd_d[s, j])
                    if j < 2 and p1_marker is not None:
                        # keep the big Wd prefetches out of the startup
                        # window where they compete with first-needed DMAs
                        add_dep_helper(p1_marker.ins, dd.ins, sync=False,
                                       reason="delay wd prefetch")
                    wdt.append(d)
                for (s, c0, n, lc) in units:
                    pd = ps2.tile([P, NT], f32, tag="pd", name="pd")[:, :n]
                    for i in range(IK):
                        nc.tensor.matmul(pd, wdt[s][:, i, :],
                                         hid[s][:, i, lc:lc + n],
                                         start=(i == 0), stop=(i == IK - 1))
                    yt = ypool.tile([P, NT], bf, tag="yt", name="yt")[:, :n]
                    nc.vector.tensor_copy(yt, pd)
                    nc.sync.dma_start(y_d[j, :, c0:c0 + n], yt)

    nc.compile()
    return nc


def _pack_gateup(w):        # [H, I] -> [IK, P(ki), HK, P(ii)] contiguous
    return np.ascontiguousarray(
        w.reshape(HK, P, IK, P).transpose(2, 1, 0, 3))


def _pack_down(w):          # [I, H] -> [HK, P(ii), IK, P(jj)] contiguous
    return np.ascontiguousarray(
        w.reshape(IK, P, HK, P).transpose(2, 1, 0, 3))


def kernel(x, Wr, Wg, Wu, Wd):
    global LAST_EXEC_NS
    x = np.asarray(x)
    B, S, _ = x.shape
    T = B * S
    xf = np.asarray(x, dtype=np.float32).reshape(T, H)

    idx, w = _route(x, Wr)

    # per-expert token lists and combine weights
    toks, cws = [], []
    for e in range(E):
        m = idx == e
        te = np.nonzero(m.any(axis=1))[0]
        toks.append(te)
        cws.append((w * m).sum(axis=1)[te].astype(np.float32))
    counts = np.array([len(t) for t in toks])

    # Scheme "o" (original): biggest expert split over all 8 B-bins, the
    # other 8 each own an A-bin. Scheme "n": biggest expert over 7 B-bins,
    # 2nd-biggest gets core 7's A-bin plus its B-bin for the overflow --
    # C drops by ~9 tokens for these counts. Pick whichever packs tighter.
    order = [int(e) for e in np.argsort(-counts)]
    big1, big2, rest = order[0], order[1], order[2:]
    CA_n = max(2, int(counts[rest].max()))
    CB_n = max(2, int(max(-(-int(counts[big1]) // (NCORES - 1)),
                          int(counts[big2]) - CA_n)))
    CA_o = max(2, int(max(counts[e] for e in order[1:])))
    CB_o = max(2, int(-(-int(counts[big1]) // NCORES)))

    def chunk(e, lo, hi):
        return (e, toks[e][lo:hi], cws[e][lo:hi])

    if CA_n + CB_n < CA_o + CB_o:
        CA, CB = CA_n, CB_n
        slotA = [chunk(rest[c], 0, CA) for c in range(NCORES - 1)]
        slotA.append(chunk(big2, 0, CA))
        slotB = [chunk(big1, c * CB, (c + 1) * CB)
                 for c in range(NCORES - 1)]
        slotB.append(chunk(big2, CA, CA + CB))
    else:
        CA, CB = CA_o, CB_o
        slotA = [chunk(order[1 + c], 0, CA) for c in range(NCORES)]
        slotB = [chunk(big1, c * CB, (c + 1) * CB) for c in range(NCORES)]
    C = CA + CB

    key = (CA, CB)
    if key not in _PROGRAM_CACHE:
        _PROGRAM_CACHE[key] = _build_program(CA, CB)
    nc = _PROGRAM_CACHE[key]

    packs = {}

    def packs_of(e):
        if e not in packs:
            packs[e] = (_pack_gateup(np.asarray(Wg[e], dtype=BF16)),
                        _pack_gateup(np.asarray(Wu[e], dtype=BF16)),
                        _pack_down(np.asarray(Wd[e], dtype=BF16)))
        return packs[e]

    in_maps = []
    for c in range(NCORES):
        (ea, ta, _), (eb, tb_c, _) = slotA[c], slotB[c]
        pa, pb = packs_of(ea), packs_of(eb)
        xt = np.zeros((H, C), dtype=BF16)
        if len(ta):
            xt[:, :len(ta)] = xf[ta].T
        if len(tb_c):
            xt[:, CA:CA + len(tb_c)] = xf[tb_c].T
        in_maps.append({
            "xt": np.ascontiguousarray(xt.reshape(HK, P, C)),
            "wg": np.stack([pa[0], pb[0]]),
            "wu": np.stack([pa[1], pb[1]]),
            "wd": np.stack([pa[2], pb[2]]),
        })

    res = run_bass_kernel_spmd(nc, in_maps, core_ids=list(range(NCORES)))
    LAST_EXEC_NS = res.exec_time_ns

    out = np.zeros((T, H), dtype=np.float32)
    for c in range(NCORES):
        y = np.asarray(res.results[c]["y"], dtype=np.float32)
        y = y.reshape(H, C).T                 # [C, H]
        (_, ta, wa), (_, tb_c, wb) = slotA[c], slotB[c]
        if len(ta):
            out[ta] += y[:len(ta)] * wa[:, None]
        if len(tb_c):
            out[tb_c] += y[CA:CA + len(tb_c)] * wb[:, None]

    return out.reshape(B, S, H)
